# revision 1
# baseline (speedup 1.0000x reference)
"""3-layer GAT on 8 trn2 NeuronCores.

Strategy (graph/data parallel per sharding hint):
  - Nodes are assigned to 8 cores x 49 blocks x 128 slots (degree-balanced
    LPT bin packing) -> permuted node order; "table row" = block*128 + slot.
  - Per layer: each core transforms its own node shard with
    rhs = [W | W@as | W@ad] (alpha terms folded into the matmul), writes a
    table shard [6272, F+2H(padded)], AllGather -> full table on every core.
  - Aggregation: per dst-block of 128 nodes, edges (dst-sorted) are packed
    into 128-edge tiles; a dma_gather fetches table rows for the tile's
    sources; a one-hot "scatter matrix" matmul accumulates both the
    s_e-weighted feature sum and the softmax denominator into PSUM.
    (Softmax max-shift is skipped: logits are O(1) so exp is safe, and the
    result is mathematically identical.)
  - int16 gather indices: table split into lo rows [0,32768) and hi rows
    [17408,50176); per-block edges are balanced between the (overlapping)
    windows so each side fits 9 tiles of 128.
  - Layer 2 output is column-summed per core (masked for pad slots); the
    final mean + linear head run on host.
"""

import os
import numpy as np

# ---------------- problem constants (must match reference) ----------------
N = 50000
E = 800000
IN_C = 128
HID = 64
HEADS = 4
OUT_C = 64
F1 = HEADS * HID  # 256

# ---------------- sharding geometry ----------------
NCORES = 8
NB = 49           # dst blocks per core
BS = 128          # dst slots per block
NPC = NB * BS     # 6272 nodes per core
RTOT = NCORES * NPC  # 50176 table rows
TL = 9            # tiles per kind (lo/hi)
KE = TL * 128     # 1152 edge slots per (block, kind)
LO_LIM = 32768    # lo window rows [0, LO_LIM)
HI_BASE = 17408   # hi window rows [HI_BASE, HI_BASE+32768)
NKCOLS = KE // 16  # 72 idx columns per (block, kind)

USE_BF16 = os.environ.get("GAT_BF16", "0") == "1"

if USE_BF16:
    import ml_dtypes
    TB_NP = ml_dtypes.bfloat16
    EL01 = 384     # table elems/row layer0/1 (256 h + 4 as + 4 ad + pad)
    EL2 = 128      # table elems/row layer2 (64 h + 1 as + 1 ad + pad)
else:
    TB_NP = np.float32
    EL01 = 320
    EL2 = 128


# ---------------- host preprocessing ----------------

def preprocess(edge_index):
    """Node->(core,block,slot) assignment and per-core edge tile arrays.

    Returns dict with:
      row:   [N] table row of each node
      xperm: [RTOT] node id occupying each table row (-1 for pad slots)
      idx16: [NCORES,128,NB*2*NKCOLS] int16 wrapped gather indices
      dstc:  [NCORES,128,NB*2*TL] f32 dst_local per edge slot (col layout, -1 pad)
      dstr:  [NCORES,128,KE] f32 dst_local (row layout; partition=block*2+kind)
      maskc: [NCORES,128,NB] f32 1.0 for real-node slots
    """
    import heapq

    src = np.concatenate([np.asarray(edge_index[0]), np.arange(N, dtype=np.int64)])
    dst = np.concatenate([np.asarray(edge_index[1]), np.arange(N, dtype=np.int64)])
    deg = np.bincount(dst, minlength=N)

    nblocks = NCORES * NB
    order = np.argsort(-deg, kind="stable")
    heap = [(0, b) for b in range(nblocks)]
    heapq.heapify(heap)
    slots_used = np.zeros(nblocks, np.int64)
    node_block = np.empty(N, np.int64)
    node_slot = np.empty(N, np.int64)
    for n in order:
        popped = []
        while True:
            load, b = heapq.heappop(heap)
            if slots_used[b] < BS:
                break
            popped.append((load, b))
        node_block[n] = b
        node_slot[n] = slots_used[b]
        slots_used[b] += 1
        heapq.heappush(heap, (load + int(deg[n]), b))
        # blocks that were full stay out of the heap

    row = node_block * BS + node_slot  # table row per node

    xperm = np.full(RTOT, -1, np.int64)
    xperm[row] = np.arange(N)

    erow = row[src]          # gather row per edge
    eblk = node_block[dst]   # destination block per edge
    eslot = node_slot[dst]   # dst_local per edge

    idx16 = np.zeros((NCORES, 128, NB * 2 * NKCOLS), np.int16)
    dstc = np.full((NCORES, 128, NB * 2 * TL), -1.0, np.float32)
    dstr = np.zeros((NCORES, 128, KE), np.float32)
    maskc = np.zeros((NCORES, 128, NB), np.float32)

    order_e = np.argsort(eblk, kind="stable")
    bounds = np.searchsorted(eblk[order_e], np.arange(nblocks + 1))

    for b in range(nblocks):
        c, bl = divmod(b, NB)
        es = order_e[bounds[b]:bounds[b + 1]]
        r_ = erow[es]
        dl = eslot[es]
        lo_f = r_ < HI_BASE
        hi_f = r_ >= LO_LIM
        flex = ~lo_f & ~hi_f
        n_lo = int(lo_f.sum())
        n_hi = int(hi_f.sum())
        n_fx = int(flex.sum())
        tot = n_lo + n_hi + n_fx
        assert tot <= 2 * KE, f"block {b} has {tot} edges > {2*KE}"
        # send flex edges to lo until lo reaches ceil(tot/2) (capped at KE)
        add_lo = min(n_fx, max(0, min(KE, (tot + 1) // 2) - n_lo))
        if n_hi + (n_fx - add_lo) > KE:
            add_lo = n_fx - (KE - n_hi)
        assert 0 <= add_lo <= n_fx
        fx_idx = np.nonzero(flex)[0]
        sel_lo = np.zeros(len(es), bool)
        sel_lo[lo_f] = True
        sel_lo[fx_idx[:add_lo]] = True
        sel_hi = ~sel_lo
        assert sel_lo.sum() <= KE and sel_hi.sum() <= KE, (
            b, sel_lo.sum(), sel_hi.sum())

        for kind, sel, base in ((0, sel_lo, 0), (1, sel_hi, HI_BASE)):
            rr = r_[sel]
            dd = dl[sel]
            o = np.argsort(rr, kind="stable")  # DMA locality
            rr = rr[o]
            dd = dd[o]
            k = len(rr)
            rel = np.zeros(KE, np.int64)
            rel[:k] = rr - base
            dloc = np.full(KE, -1.0, np.float32)
            dloc[:k] = dd.astype(np.float32)
            assert rel.min() >= 0 and rel.max() < 32768
            # wrapped idx: index i -> [i % 16, i // 16]
            w = rel.reshape(NKCOLS, 16).T.astype(np.int16)  # [16, NKCOLS]
            cbase = (bl * 2 + kind) * NKCOLS
            idx16[c, :, cbase:cbase + NKCOLS] = np.tile(w, (8, 1))
            # col layout: col bl*2*TL + kind*TL + t, partition p = edge t*128+p
            tcol = bl * 2 * TL + kind * TL
            dstc[c, :, tcol:tcol + TL] = dloc.reshape(TL, 128).T
            # row layout: partition bl*2+kind
            dstr[c, bl * 2 + kind, :] = dloc

        # mask of real slots
        used = slots_used[b]
        maskc[c, :used, bl] = 1.0

    return dict(row=row, xperm=xperm, idx16=idx16, dstc=dstc, dstr=dstr,
                maskc=maskc, deg=deg, node_block=node_block,
                node_slot=node_slot)


def host_weights(inputs):
    """Extended weight matrices with folded attention vectors."""
    def ext(W, a_s, a_d, heads):
        # Was[k, h] = sum_c W[k, h*HID+c] * a_s[h, c]
        Wh = W.reshape(W.shape[0], heads, HID)
        Was = np.einsum("khc,hc->kh", Wh, a_s)
        Wad = np.einsum("khc,hc->kh", Wh, a_d)
        return np.concatenate([W, Was, Wad], axis=1).astype(np.float32)

    W0e = ext(np.asarray(inputs["W0"], np.float32),
              np.asarray(inputs["a0s"], np.float32),
              np.asarray(inputs["a0d"], np.float32), HEADS)      # [128, 264]
    W1e = ext(np.asarray(inputs["W1"], np.float32),
              np.asarray(inputs["a1s"], np.float32),
              np.asarray(inputs["a1d"], np.float32), HEADS)      # [256, 264]
    W2e = ext(np.asarray(inputs["W2"], np.float32),
              np.asarray(inputs["a2s"], np.float32),
              np.asarray(inputs["a2d"], np.float32), 1)          # [256, 66]
    return W0e, W1e, W2e


def build_core_inputs(inputs, pp):
    """Per-core in_maps for run_bass_kernel_spmd."""
    x = np.asarray(inputs["x"], np.float32)
    W0e, W1e, W2e = host_weights(inputs)
    b0 = np.asarray(inputs["b0"], np.float32)
    b1 = np.asarray(inputs["b1"], np.float32)
    b2 = np.asarray(inputs["b2"], np.float32)

    iota_row = np.tile(np.arange(128, dtype=np.float32), (128, 1))
    iota_col = np.arange(128, dtype=np.float32).reshape(128, 1)
    ones1 = np.ones((1, 128), np.float32)
    ident = np.eye(128, dtype=np.float32)

    consts = dict(
        w0e=W0e,                                    # [128, 264]
        w1e=W1e.reshape(2, 128, F1 + 2 * HEADS),    # [2, 128, 264]
        w2e=W2e.reshape(2, 128, HID + 2),           # [2, 128, 66]
        b0r=np.tile(b0, (128, 1)).astype(np.float32),
        b1r=np.tile(b1, (128, 1)).astype(np.float32),
        b2r=np.tile(b2, (128, 1)).astype(np.float32),
        iota_row=iota_row, iota_col=iota_col, ones1=ones1, ident=ident,
    )

    in_maps = []
    for c in range(NCORES):
        # xTb[b] = x[nodes of (c,b)].T : [128 feats, 128 slots]
        xtb = np.zeros((NB, IN_C, BS), np.float32)
        rows = np.arange(c * NPC, (c + 1) * NPC)
        nodes = pp["xperm"][rows].reshape(NB, BS)
        for b in range(NB):
            nb = nodes[b]
            valid = nb >= 0
            if valid.any():
                xtb[b][:, valid] = x[nb[valid]].T
        m = dict(
            xtb=xtb,
            idx16=pp["idx16"][c],
            dstc=pp["dstc"][c],
            dstr=pp["dstr"][c],
            maskc=pp["maskc"][c],
            **consts,
        )
        in_maps.append(m)
    return in_maps


# ---------------- numpy emulation of the device data path ----------------

def _emulate_layer(tables_in, pp, We, brep, heads, F_out, relu, el):
    """tables_in: hT equivalent — full node-major feature mat [RTOT, F_in].
    Returns (out [RTOT, F_out] node-major post-activation, table [RTOT, el])."""
    Fi = We.shape[0]
    Fo = F_out * 1
    # transform (all rows; pad rows produce garbage but are never gathered)
    tb = tables_in @ We  # [RTOT, Fo + 2*heads]
    table = np.zeros((RTOT, el), TB_NP)
    table[:, :Fo + 2 * heads] = tb.astype(TB_NP)
    ad_all = tb[:, Fo + heads:Fo + 2 * heads]  # [RTOT, heads]

    out = np.zeros((RTOT, Fo), np.float32)
    for c in range(NCORES):
        for bl in range(NB):
            rbase = c * NPC + bl * BS
            agg = np.zeros((BS, Fo), np.float32)
            den = np.zeros((BS, heads), np.float32)
            for kind in range(2):
                base = 0 if kind == 0 else HI_BASE
                cbase = (bl * 2 + kind) * NKCOLS
                w = pp["idx16"][c][:16, cbase:cbase + NKCOLS]
                rel = w.T.reshape(-1).astype(np.int64)  # unwrap
                rows = rel + base
                g = np.asarray(table[rows], np.float32)  # [KE, el]
                dl = pp["dstr"][c][bl * 2 + kind].astype(np.int64)  # -1 pads
                valid = dl >= 0
                a_s = g[:, Fo:Fo + heads]
                a_d = np.where(valid[:, None], ad_all[rbase + dl], 0.0)
                z = a_s + a_d
                s = np.exp(np.maximum(z, 0.2 * z)).astype(np.float32)
                hsc = (g[:, :Fo].reshape(KE, heads, HID)
                       * s[:, :, None]).astype(TB_NP).astype(np.float32)
                hsc = hsc.reshape(KE, Fo)
                np.add.at(agg, dl[valid], hsc[valid])
                np.add.at(den, dl[valid], s[valid])
            o = agg.reshape(BS, heads, HID) / (den + 1e-16)[:, :, None]
            o = o.reshape(BS, Fo) + brep[0]
            if relu:
                o = np.maximum(o, 0.0)
            out[rbase:rbase + BS] = o
    return out


def emulate(inputs, pp=None):
    """Full numpy emulation; returns [1, OUT_C]."""
    if pp is None:
        pp = preprocess(np.asarray(inputs["edge_index"]))
    x = np.asarray(inputs["x"], np.float32)
    W0e, W1e, W2e = host_weights(inputs)
    h = np.zeros((RTOT, IN_C), np.float32)
    valid = pp["xperm"] >= 0
    h[valid] = x[pp["xperm"][valid]]

    b0r = np.tile(np.asarray(inputs["b0"], np.float32), (1, 1))
    b1r = np.tile(np.asarray(inputs["b1"], np.float32), (1, 1))
    b2r = np.tile(np.asarray(inputs["b2"], np.float32), (1, 1))

    h0 = _emulate_layer(h, pp, W0e, b0r, HEADS, F1, True, EL01)
    h1 = _emulate_layer(h0, pp, W1e, b1r, HEADS, F1, True, EL01)
    h2 = _emulate_layer(h1, pp, W2e, b2r, 1, HID, False, EL2)

    g = h2[valid].sum(axis=0, keepdims=True) / N
    return (g @ np.asarray(inputs["hw"], np.float32)
            + np.asarray(inputs["hb"], np.float32)).astype(np.float32)


# ---------------- device kernel ----------------

_BUILT = None


def build_kernel(upto=99):
    import concourse.bacc as bacc
    import concourse.bass as bass
    import concourse.mybir as mybir
    import concourse.tile as tile
    from concourse import library_config

    f32 = mybir.dt.float32
    tb_dt = mybir.dt.bfloat16 if USE_BF16 else mybir.dt.float32
    i16 = mybir.dt.int16
    Alu = mybir.AluOpType
    Act = mybir.ActivationFunctionType

    nc = bacc.Bacc("TRN2", target_bir_lowering=False, debug=False,
                   num_devices=NCORES)

    # ---- I/O ----
    xtb_d = nc.dram_tensor("xtb", [NB, IN_C, BS], f32, kind="ExternalInput")
    idx16_d = nc.dram_tensor("idx16", [128, NB * 2 * NKCOLS], i16,
                             kind="ExternalInput")
    dstc_d = nc.dram_tensor("dstc", [128, NB * 2 * TL], f32,
                            kind="ExternalInput")
    dstr_d = nc.dram_tensor("dstr", [128, KE], f32, kind="ExternalInput")
    maskc_d = nc.dram_tensor("maskc", [128, NB], f32, kind="ExternalInput")
    w0e_d = nc.dram_tensor("w0e", [IN_C, F1 + 2 * HEADS], f32,
                           kind="ExternalInput")
    w1e_d = nc.dram_tensor("w1e", [2, 128, F1 + 2 * HEADS], f32,
                           kind="ExternalInput")
    w2e_d = nc.dram_tensor("w2e", [2, 128, HID + 2], f32,
                           kind="ExternalInput")
    b0r_d = nc.dram_tensor("b0r", [128, F1], f32, kind="ExternalInput")
    b1r_d = nc.dram_tensor("b1r", [128, F1], f32, kind="ExternalInput")
    b2r_d = nc.dram_tensor("b2r", [128, HID], f32, kind="ExternalInput")
    iota_row_d = nc.dram_tensor("iota_row", [128, 128], f32,
                                kind="ExternalInput")
    iota_col_d = nc.dram_tensor("iota_col", [128, 1], f32,
                                kind="ExternalInput")
    ones1_d = nc.dram_tensor("ones1", [1, 128], f32, kind="ExternalInput")
    ident_d = nc.dram_tensor("ident", [128, 128], f32, kind="ExternalInput")
    out_d = nc.dram_tensor("out_part", [1, OUT_C], f32, kind="ExternalOutput")
    debug = os.environ.get("GAT_DEBUG", "0") == "1"
    if debug:
        dmp_tb = nc.dram_tensor("dmp_tb", [NPC, EL01], f32,
                                kind="ExternalOutput")
        dmp_h = nc.dram_tensor("dmp_h", [NPC, F1], f32, kind="ExternalOutput")
        dmp_den = nc.dram_tensor("dmp_den", [NPC, HEADS], f32,
                                 kind="ExternalOutput")
        dmp_tmp = nc.dram_tensor("dmp_tmp", [128, TL, F1], f32,
                                 kind="ExternalOutput")
        dmp_agg = nc.dram_tensor("dmp_agg", [128, F1 + HEADS], f32,
                                 kind="ExternalOutput")
        dmp_g = nc.dram_tensor("dmp_g", [128, TL, EL01], f32,
                               kind="ExternalOutput")
        dmp_s = nc.dram_tensor("dmp_s", [128, TL * HEADS], f32,
                               kind="ExternalOutput")

    # internal DRAM
    shard01 = nc.dram_tensor("shard01", [NPC, EL01], tb_dt)
    table01 = nc.dram_tensor("table01", [RTOT, EL01], tb_dt)
    shard2 = nc.dram_tensor("shard2", [NPC, EL2], tb_dt)
    table2 = nc.dram_tensor("table2", [RTOT, EL2], tb_dt)

    rg = [list(range(NCORES))]

    with tile.TileContext(nc) as tc:
        with (
            tc.tile_pool(name="const", bufs=1) as cpool,
            tc.tile_pool(name="big", bufs=1) as bigpool,
            tc.tile_pool(name="work", bufs=3) as wpool,
            tc.tile_pool(name="gather", bufs=3) as gpool,
            tc.tile_pool(name="small", bufs=4) as spool,
            tc.tile_pool(name="psum", bufs=2, space="PSUM") as ppool,
            tc.tile_pool(name="psum1", bufs=1, space="PSUM") as ppool1,
        ):
            # ---- load constants ----
            def load_const(tag, dram, shape, dtype=f32, view=None):
                t = cpool.tile(shape, dtype, tag=tag)
                nc.sync.dma_start(out=t[:], in_=view if view is not None
                                  else dram[:])
                return t

            w0e_s = load_const("w0e", w0e_d, [IN_C, F1 + 2 * HEADS])
            w1e_s = load_const("w1e", w1e_d, [128, 2, F1 + 2 * HEADS],
                               view=w1e_d[:].rearrange("c p j -> p c j"))
            w2e_s = load_const("w2e", w2e_d, [128, 2, HID + 2],
                               view=w2e_d[:].rearrange("c p j -> p c j"))
            b0r_s = load_const("b0r", b0r_d, [128, F1])
            b1r_s = load_const("b1r", b1r_d, [128, F1])
            b2r_s = load_const("b2r", b2r_d, [128, HID])
            iota_row_s = load_const("iota_row", iota_row_d, [128, 128])
            iota_col_s = load_const("iota_col", iota_col_d, [128, 1])
            ones1_s = load_const("ones1", ones1_d, [1, 128])
            ident_s = load_const("ident", ident_d, [128, 128])
            idx16_s = load_const("idx16", idx16_d,
                                 [128, NB * 2 * NKCOLS], i16)
            dstc_s = load_const("dstc", dstc_d, [128, NB * 2 * TL])
            maskc_s = load_const("maskc", maskc_d, [128, NB])

            nc.gpsimd.load_library(library_config.mlp)

            hT = bigpool.tile([128, 2, NPC], f32, tag="hT")

            def transform(layer):
                """Own-shard transform -> shard DRAM + ad_all SBUF."""
                heads = 1 if layer == 2 else HEADS
                Fo = HID if layer == 2 else F1
                ncols = Fo + 2 * heads
                el = EL2 if layer == 2 else EL01
                shard = shard2 if layer == 2 else shard01
                ad_all = spool.tile([128, NB * heads], f32, tag="ad_all")
                for b in range(NB):
                    ps = ppool.tile([128, 512], f32, tag="agg", space="PSUM")
                    if layer == 0:
                        xb = wpool.tile([IN_C, BS], f32, tag="xtb")
                        nc.sync.dma_start(out=xb[:], in_=xtb_d[b])
                        nc.tensor.matmul(out=ps[:, :ncols], lhsT=xb[:],
                                         rhs=w0e_s[:], start=True, stop=True)
                    else:
                        we = w1e_s if layer == 1 else w2e_s
                        for k2 in range(2):
                            nc.tensor.matmul(
                                out=ps[:, :ncols],
                                lhsT=hT[:, k2, b * BS:(b + 1) * BS],
                                rhs=we[:, k2, :],
                                start=(k2 == 0), stop=(k2 == 1))
                    tb = wpool.tile([128, el], tb_dt, tag="tbout")
                    nc.vector.tensor_copy(out=tb[:, :ncols],
                                          in_=ps[:, :ncols])
                    nc.vector.tensor_copy(
                        out=ad_all[:, b * heads:(b + 1) * heads],
                        in_=ps[:, Fo + heads:Fo + 2 * heads])
                    nc.sync.dma_start(out=shard[b * BS:(b + 1) * BS, :],
                                      in_=tb[:])
                    if debug and layer == 0:
                        nc.sync.dma_start(
                            out=dmp_tb[b * BS:(b + 1) * BS, :], in_=tb[:])
                return ad_all

            def allgather(layer):
                shard = shard2 if layer == 2 else shard01
                table = table2 if layer == 2 else table01
                nc.gpsimd.collective_compute(
                    "AllGather", mybir.AluOpType.bypass,
                    replica_groups=rg, ins=[shard[:].opt()],
                    outs=[table[:].opt()])

            def aggregate(layer, ad_all):
                sub = int(os.environ.get("GAT_AGG_SUB", "99"))
                heads = 1 if layer == 2 else HEADS
                Fo = HID if layer == 2 else F1
                el = EL2 if layer == 2 else EL01
                table = table2 if layer == 2 else table01
                brep = (b2r_s, b1r_s, b1r_s)[0] if False else (
                    b0r_s if layer == 0 else (b1r_s if layer == 1 else b2r_s))
                views = [table[0:LO_LIM, :], table[HI_BASE:HI_BASE + 32768, :]]
                if layer == 2:
                    psum_sum = ppool1.tile([1, OUT_C], f32, tag="sum",
                                          space="PSUM")
                for b in range(NB):
                    pagg = ppool.tile([128, Fo], f32, tag="agg",
                                      space="PSUM")
                    pden = ppool.tile([128, heads], f32, tag="den_ps",
                                      space="PSUM")
                    for kind in range(2):
                        bk = b * 2 + kind
                        g = gpool.tile([128, TL, el], tb_dt, tag="g")
                        nc.gpsimd.dma_gather(
                            g[:], views[kind],
                            idx16_s[:, bk * NKCOLS:(bk + 1) * NKCOLS],
                            KE, KE, el, single_packet=False)
                        if sub < 2:
                            continue
                        # one-hot M [128e, TL*128d]
                        M = wpool.tile([128, KE], tb_dt, tag="M")
                        tcol = b * 2 * TL + kind * TL
                        nc.vector.tensor_tensor(
                            out=M[:].rearrange("p (t d) -> p t d", t=TL),
                            in0=dstc_s[:, tcol:tcol + TL].unsqueeze(-1)
                                .broadcast_to([128, TL, 128]),
                            in1=iota_row_s[:].unsqueeze(1)
                                .broadcast_to([128, TL, 128]),
                            op=Alu.is_equal)
                        if sub < 3:
                            continue
                        # M_T [128d, TL*128e] via replicated-row outer product
                        MT = wpool.tile([128, KE], f32, tag="MT")
                        dr = spool.tile([1, KE], f32, tag="dr")
                        nc.sync.dma_start(out=dr[:], in_=dstr_d[bk:bk + 1, :])
                        for o, wdt in ((0, 512), (512, 512), (1024, 128)):
                            pr = ppool1.tile([128, 512], f32, tag="rep",
                                            space="PSUM")
                            nc.tensor.matmul(out=pr[:, :wdt],
                                             lhsT=ones1_s[:],
                                             rhs=dr[:, o:o + wdt],
                                             start=True, stop=True)
                            nc.vector.tensor_tensor(
                                out=MT[:, o:o + wdt], in0=pr[:, :wdt],
                                in1=iota_col_s[:]
                                    .broadcast_to([128, wdt]),
                                op=Alu.is_equal)
                        if sub < 4:
                            continue
                        # ad per edge via M_T @ ad_block
                        pad_ = ppool1.tile([128, TL * heads], f32, tag="adp",
                                          space="PSUM")
                        for t in range(TL):
                            nc.tensor.matmul(
                                out=pad_[:, t * heads:(t + 1) * heads],
                                lhsT=MT[:, t * 128:(t + 1) * 128],
                                rhs=ad_all[:, b * heads:(b + 1) * heads],
                                start=True, stop=True)
                        if sub < 5:
                            continue
                        # z = as + ad ; s = exp(max(z, 0.2 z))
                        z = spool.tile([128, TL * heads], f32, tag="z")
                        nc.vector.tensor_tensor(
                            out=z[:].rearrange("p (t h) -> p t h", t=TL),
                            in0=g[:, :, Fo:Fo + heads],
                            in1=pad_[:].rearrange("p (t h) -> p t h", t=TL),
                            op=Alu.add)
                        z2 = spool.tile([128, TL * heads], f32, tag="z2")
                        nc.vector.tensor_scalar(out=z2[:], in0=z[:],
                                                scalar1=0.2, scalar2=None,
                                                op0=Alu.mult)
                        zm = spool.tile([128, TL * heads], f32, tag="zm")
                        nc.vector.tensor_tensor(out=zm[:], in0=z[:],
                                                in1=z2[:], op=Alu.max)
                        s_t = spool.tile([128, TL * heads], tb_dt, tag="s")
                        nc.scalar.activation(s_t[:], zm[:], Act.Exp)
                        if sub < 6:
                            continue
                        # tmp = g[:, :, :Fo] * s (broadcast over HID),
                        # one 3D op per head (4D broadcast APs miscompute)
                        tmp = wpool.tile([128, TL, Fo], tb_dt, tag="tmp")
                        sv = s_t[:].rearrange("p (t h) -> p t h", t=TL)
                        for hh in range(heads):
                            nc.vector.tensor_tensor(
                                out=tmp[:, :, hh * HID:(hh + 1) * HID],
                                in0=g[:, :, hh * HID:(hh + 1) * HID],
                                in1=sv[:, :, hh:hh + 1]
                                    .broadcast_to([128, TL, HID]),
                                op=Alu.mult)
                        if debug and layer == 0 and b == 0 and kind == 0:
                            nc.sync.dma_start(out=dmp_tmp[:], in_=tmp[:])
                            nc.sync.dma_start(out=dmp_g[:], in_=g[:])
                            nc.sync.dma_start(out=dmp_s[:], in_=s_t[:])
                        if sub < 7:
                            continue
                        # accumulate
                        for t in range(TL):
                            first = (kind == 0 and t == 0)
                            last = (kind == 1 and t == TL - 1)
                            nc.tensor.matmul(
                                out=pagg[:],
                                lhsT=M[:, t * 128:(t + 1) * 128],
                                rhs=tmp[:, t, :],
                                start=first, stop=last)
                            nc.tensor.matmul(
                                out=pden[:],
                                lhsT=M[:, t * 128:(t + 1) * 128],
                                rhs=s_t[:, t * heads:(t + 1) * heads],
                                start=first, stop=last)
                    if sub < 8:
                        continue
                    # epilogue
                    if debug and layer == 0 and b == 0:
                        aggc = wpool.tile([128, F1 + HEADS], f32, tag="aggc")
                        nc.vector.tensor_copy(out=aggc[:, :F1], in_=pagg[:])
                        nc.vector.tensor_copy(out=aggc[:, F1:], in_=pden[:])
                        nc.sync.dma_start(out=dmp_agg[:], in_=aggc[:])
                    den = spool.tile([128, heads], f32, tag="den")
                    nc.vector.tensor_scalar(out=den[:],
                                            in0=pden[:],
                                            scalar1=1e-16, scalar2=None,
                                            op0=Alu.add)
                    rec = spool.tile([128, heads], f32, tag="rec")
                    nc.vector.reciprocal(out=rec[:], in_=den[:])
                    if debug and layer == 0:
                        nc.sync.dma_start(
                            out=dmp_den[b * BS:(b + 1) * BS, :], in_=den[:])
                    o1 = wpool.tile([128, Fo], f32, tag="o1")
                    nc.vector.tensor_tensor(
                        out=o1[:].rearrange("p (h f) -> p h f", h=heads),
                        in0=pagg[:].rearrange("p (h f) -> p h f",
                                              h=heads),
                        in1=rec[:].unsqueeze(-1)
                            .broadcast_to([128, heads, HID]),
                        op=Alu.mult)
                    o2 = wpool.tile([128, Fo], f32, tag="o2")
                    nc.vector.tensor_tensor(out=o2[:], in0=o1[:],
                                            in1=brep[:, :Fo], op=Alu.add)
                    if layer == 2:
                        nc.tensor.matmul(out=psum_sum[:],
                                         lhsT=maskc_s[:, b:b + 1],
                                         rhs=o2[:], start=(b == 0),
                                         stop=(b == NB - 1))
                    else:
                        o3 = wpool.tile([128, Fo], f32, tag="o3")
                        nc.scalar.activation(o3[:], o2[:], Act.Relu)
                        if debug and layer == 0:
                            nc.sync.dma_start(
                                out=dmp_h[b * BS:(b + 1) * BS, :], in_=o3[:])
                        for k2 in range(2):
                            pt = ppool1.tile([128, 128], f32, tag="tp",
                                            space="PSUM")
                            nc.tensor.transpose(
                                pt[:], o3[:, k2 * 128:(k2 + 1) * 128],
                                ident_s[:])
                            nc.vector.tensor_copy(
                                out=hT[:, k2, b * BS:(b + 1) * BS],
                                in_=pt[:])
                if layer == 2:
                    osb = spool.tile([1, OUT_C], f32, tag="osb")
                    nc.vector.tensor_copy(out=osb[:], in_=psum_sum[:])
                    nc.sync.dma_start(out=out_d[:], in_=osb[:])

            stage = 0
            for layer in range(3):
                if stage >= upto:
                    break
                ad_all = transform(layer)
                stage += 1
                if stage >= upto:
                    break
                allgather(layer)
                stage += 1
                if stage >= upto:
                    break
                aggregate(layer, ad_all)
                stage += 1

    nc.compile()
    return nc


def _get_built():
    global _BUILT
    if _BUILT is None:
        _BUILT = build_kernel(upto=int(os.environ.get("GAT_UPTO", "99")))
    return _BUILT


def kernel(**inputs) -> np.ndarray:
    from concourse.bass_utils import run_bass_kernel_spmd

    pp = preprocess(np.asarray(inputs["edge_index"]))
    in_maps = build_core_inputs(inputs, pp)
    nc = _get_built()
    res = run_bass_kernel_spmd(nc, in_maps, core_ids=list(range(NCORES)))
    parts = np.stack([r["out_part"][0] for r in res.results])  # [8, 64]
    g = parts.sum(axis=0, keepdims=True) / N
    out = (g @ np.asarray(inputs["hw"], np.float32)
           + np.asarray(inputs["hb"], np.float32)).astype(np.float32)
    return out



# revision 7
# speedup vs baseline: 1.3086x; 1.3086x over previous
"""3-layer GAT on 8 trn2 NeuronCores.

Strategy (graph/data parallel per sharding hint):
  - Nodes assigned to 8 cores x 49 blocks x 128 slots (degree-balanced LPT
    packing) -> permuted node order; table row = block*128 + slot.
  - Per layer, per node block: transform with rhs = [W | W@as | W@ad] (alpha
    terms folded into the matmul) -> bf16 table shard [6272, 384]; AllGather
    (Shared output) -> full table on every core.
  - Aggregation per dst block: edges (minus self loops, dst-sorted) packed
    into <=1152-edge windows (lo rows [0,32768), hi rows [17408,50176)) so
    int16 gather indices fit; dma_gather fetches the src rows; one-hot
    scatter matrices M [edge,dst] / MT [dst,edge] built on-device feed
    matmuls for the per-edge ad term and the fused (feature, denom)
    accumulation in PSUM.  Self-loop contributions are computed locally
    (own-shard rows) and never gathered.
  - Softmax max-shift skipped (logits O(1), exp safe; mathematically equal).
  - Next layer's transform is interleaved into the aggregation loop
    (block-level pipeline); layer 2 reduces via a mask matmul; final mean +
    linear head on host.
"""

import os
import numpy as np
import ml_dtypes

# ---------------- problem constants (must match reference) ----------------
N = 50000
E = 800000
IN_C = 128
HID = 64
HEADS = 4
OUT_C = 64
F1 = HEADS * HID  # 256

# ---------------- sharding geometry ----------------
NCORES = 8
NB = 49            # dst blocks per core
BS = 128           # dst slots per block
NPC = NB * BS      # 6272 nodes per core
RTOT = NCORES * NPC  # 50176 table rows
TL = 9             # tiles per kind (lo/hi)
KE = TL * 128      # 1152 edge slots per (block, kind)
TL2 = 2 * TL       # tiles per block (both kinds)
KE2 = 2 * KE
LO_LIM = 32768     # lo window rows [0, LO_LIM)
HI_BASE = 17408    # hi window rows [HI_BASE, HI_BASE+32768)
NKCOLS = KE // 16  # 72 idx columns per (block, kind)

EL01 = 384         # table elems/row layers 0/1: 256 h + 4 as + 4 ad + pad
EL2 = 128          # table elems/row layer 2: 64 h + 1 as + 1 ad + pad
BF = ml_dtypes.bfloat16

GBUFS = 6          # gather tile double-buffer depth
SINGLE_PACKET = os.environ.get("GAT_SP", "0") == "1"


# ---------------- host preprocessing ----------------

def preprocess(edge_index):
    """Node->(core,block,slot) assignment and per-core edge tile arrays."""
    import heapq

    e0 = np.asarray(edge_index[0], np.int64)
    e1 = np.asarray(edge_index[1], np.int64)
    nonself = e0 != e1
    src = e0[nonself]
    dst = e1[nonself]
    # self-edge multiplicity: 1 (PyG added loop) + natural self edges
    mult = np.ones(N, np.int64)
    np.add.at(mult, e0[~nonself], 1)

    deg = np.bincount(dst, minlength=N)  # gather load per dst node

    nblocks = NCORES * NB
    order = np.argsort(-deg, kind="stable")
    heap = [(0, b) for b in range(nblocks)]
    heapq.heapify(heap)
    slots_used = np.zeros(nblocks, np.int64)
    node_block = np.empty(N, np.int64)
    node_slot = np.empty(N, np.int64)
    for n in order:
        while True:
            load, b = heapq.heappop(heap)
            if slots_used[b] < BS:
                break
        node_block[n] = b
        node_slot[n] = slots_used[b]
        slots_used[b] += 1
        heapq.heappush(heap, (load + int(deg[n]), b))

    row = node_block * BS + node_slot

    xperm = np.full(RTOT, -1, np.int64)
    xperm[row] = np.arange(N)

    erow = row[src]
    eblk = node_block[dst]
    eslot = node_slot[dst]

    idx16 = np.full((NCORES, 128, NB * 2 * NKCOLS), -1, np.int16)
    dstc = np.full((NCORES, 128, NB * 2 * TL), -1.0, np.float32)
    dstr = np.full((NCORES, NB, KE2), -1.0, np.float32)
    maskc = np.zeros((NCORES, 128, NB), np.float32)
    mselfc = np.zeros((NCORES, 128, NB), np.float32)
    cnt = np.zeros((NCORES, NB, 2), np.int64)

    order_e = np.argsort(eblk, kind="stable")
    bounds = np.searchsorted(eblk[order_e], np.arange(nblocks + 1))

    # pass 1: split lo/hi, count
    packed = {}
    for b in range(nblocks):
        c, bl = divmod(b, NB)
        es = order_e[bounds[b]:bounds[b + 1]]
        r_ = erow[es]
        dl = eslot[es]
        lo_f = r_ < HI_BASE
        hi_f = r_ >= LO_LIM
        flex = ~lo_f & ~hi_f
        n_lo = int(lo_f.sum())
        n_hi = int(hi_f.sum())
        n_fx = int(flex.sum())
        tot = n_lo + n_hi + n_fx
        assert tot <= 2 * KE, f"block {b} has {tot} edges > {2*KE}"
        add_lo = min(n_fx, max(0, min(KE, (tot + 1) // 2) - n_lo))
        if n_hi + (n_fx - add_lo) > KE:
            add_lo = n_fx - (KE - n_hi)
        assert 0 <= add_lo <= n_fx
        fx_idx = np.nonzero(flex)[0]
        sel_lo = np.zeros(len(es), bool)
        sel_lo[lo_f] = True
        sel_lo[fx_idx[:add_lo]] = True
        for kind, sel, base in ((0, sel_lo, 0), (1, ~sel_lo, HI_BASE)):
            rr = r_[sel]
            dd = dl[sel]
            o = np.argsort(rr, kind="stable")  # DMA locality
            packed[(b, kind)] = (rr[o] - base, dd[o])
            cnt[c, bl, kind] = len(rr)

    # static per-(block,kind) gather count = max over cores, 16-aligned
    reg = cnt.max(axis=0)
    reg = ((reg + 15) // 16 * 16).astype(np.int64)
    assert (reg <= KE).all()

    # pass 2: pack indices (pad to reg with row 0, -1 beyond)
    for b in range(nblocks):
        c, bl = divmod(b, NB)
        for kind in range(2):
            rel, dd = packed[(b, kind)]
            k = len(rel)
            r = int(reg[bl, kind])
            full = np.full(KE, -1, np.int64)
            full[:k] = rel
            full[k:r] = 0  # safe real row, contributes nothing (dst=-1)
            assert k == 0 or (rel.min() >= 0 and rel.max() < 32768)
            w = full.reshape(NKCOLS, 16).T.astype(np.int16)  # [16, NKCOLS]
            cbase = (bl * 2 + kind) * NKCOLS
            idx16[c, :, cbase:cbase + NKCOLS] = np.tile(w, (8, 1))
            dloc = np.full(KE, -1.0, np.float32)
            dloc[:k] = dd.astype(np.float32)
            tcol = bl * 2 * TL + kind * TL
            dstc[c, :, tcol:tcol + TL] = dloc.reshape(TL, 128).T
            dstr[c, bl, kind * KE:(kind + 1) * KE] = dloc

        used = slots_used[b]
        maskc[c, :used, bl] = 1.0
        nodes_b = np.where(node_block == b)[0]
        mselfc[c, node_slot[nodes_b], bl] = mult[nodes_b].astype(np.float32)

    return dict(row=row, xperm=xperm, idx16=idx16, dstc=dstc, dstr=dstr,
                maskc=maskc, mselfc=mselfc, cnt=cnt, reg=reg,
                node_block=node_block, node_slot=node_slot)


def host_weights(inputs):
    """Extended weight matrices with folded attention vectors."""
    def ext(W, a_s, a_d, heads):
        Wh = W.reshape(W.shape[0], heads, HID)
        Was = np.einsum("khc,hc->kh", Wh, a_s)
        Wad = np.einsum("khc,hc->kh", Wh, a_d)
        return np.concatenate([W, Was, Wad], axis=1).astype(np.float32)

    W0e = ext(np.asarray(inputs["W0"], np.float32),
              np.asarray(inputs["a0s"], np.float32),
              np.asarray(inputs["a0d"], np.float32), HEADS)      # [128, 264]
    W1e = ext(np.asarray(inputs["W1"], np.float32),
              np.asarray(inputs["a1s"], np.float32),
              np.asarray(inputs["a1d"], np.float32), HEADS)      # [256, 264]
    W2e = ext(np.asarray(inputs["W2"], np.float32),
              np.asarray(inputs["a2s"], np.float32),
              np.asarray(inputs["a2d"], np.float32), 1)          # [256, 66]
    return W0e, W1e, W2e


def build_core_inputs(inputs, pp):
    """Per-core in_maps for run_bass_kernel_spmd."""
    x = np.asarray(inputs["x"], np.float32)
    W0e, W1e, W2e = host_weights(inputs)
    b0 = np.asarray(inputs["b0"], np.float32)
    b1 = np.asarray(inputs["b1"], np.float32)
    b2 = np.asarray(inputs["b2"], np.float32)

    iota_row = np.tile(np.arange(128, dtype=np.float32), (128, 1))
    iota_col = np.arange(128, dtype=np.float32).reshape(128, 1)
    ones1 = np.ones((1, 128), np.float32)
    ident = np.eye(128, dtype=np.float32)

    consts = dict(
        w0e=W0e.astype(BF),
        w1e=W1e.reshape(2, 128, F1 + 2 * HEADS).astype(BF),
        w2e=W2e.reshape(2, 128, HID + 2).astype(BF),
        b0r=np.tile(b0, (128, 1)).astype(BF),
        b1r=np.tile(b1, (128, 1)).astype(BF),
        b2r=np.tile(b2, (128, 1)).astype(BF),
        iota_row=iota_row.astype(BF), iota_col=iota_col.astype(BF),
        ones1=ones1.astype(BF), ident=ident.astype(BF),
    )

    in_maps = []
    for c in range(NCORES):
        xtb = np.zeros((NB, IN_C, BS), np.float32)
        rows = np.arange(c * NPC, (c + 1) * NPC)
        nodes = pp["xperm"][rows].reshape(NB, BS)
        for b in range(NB):
            nb = nodes[b]
            valid = nb >= 0
            if valid.any():
                xtb[b][:, valid] = x[nb[valid]].T
        m = dict(
            xtb=xtb.astype(BF),
            idx16=pp["idx16"][c],
            dstc=pp["dstc"][c].astype(BF),
            dstr=pp["dstr"][c].astype(BF),
            maskc=pp["maskc"][c].astype(BF),
            mselfc=pp["mselfc"][c].astype(BF),
            **consts,
        )
        in_maps.append(m)
    return in_maps


# ---------------- numpy emulation of the device data path ----------------

def _emulate_layer(tables_in, pp, We, bias, heads, Fo, relu, el):
    """tables_in: node-major feature mat [RTOT, F_in] (f32).
    Returns out [RTOT, Fo] node-major post-activation."""
    ncols = Fo + 2 * heads
    tb = (tables_in.astype(BF).astype(np.float32)
          @ We.astype(BF).astype(np.float32))
    table = np.zeros((RTOT, el), BF)
    table[:, :ncols] = tb.astype(BF)
    as_all = tb[:, Fo:Fo + heads]
    ad_all = tb[:, Fo + heads:Fo + 2 * heads]

    def lrexp(z):
        return np.exp(np.maximum(z, 0.2 * z)).astype(np.float32)

    out = np.zeros((RTOT, Fo), np.float32)
    for c in range(NCORES):
        for bl in range(NB):
            rbase = c * NPC + bl * BS
            agg = np.zeros((BS, Fo), np.float32)
            den = np.zeros((BS, heads), np.float32)
            for kind in range(2):
                base = 0 if kind == 0 else HI_BASE
                cbase = (bl * 2 + kind) * NKCOLS
                w = pp["idx16"][c][:16, cbase:cbase + NKCOLS]
                rel = w.T.reshape(-1).astype(np.int64)
                r = int(pp["reg"][bl, kind])
                rows = rel[:r] + base
                g = np.asarray(table[rows], np.float32)  # [r, el]
                dl = pp["dstr"][c][bl, kind * KE:kind * KE + r].astype(np.int64)
                valid = dl >= 0
                a_s = g[:, Fo:Fo + heads]
                a_d = np.where(valid[:, None], ad_all[rbase + dl], 0.0)
                s = lrexp(a_s + a_d).astype(BF).astype(np.float32)
                hsc = (g[:, :Fo].reshape(r, heads, HID)
                       * s[:, :, None]).astype(BF).astype(np.float32)
                hsc = hsc.reshape(r, Fo)
                np.add.at(agg, dl[valid], hsc[valid])
                np.add.at(den, dl[valid], s[valid])
            # self loops
            ms = pp["mselfc"][c][:, bl]  # [BS]
            ss = lrexp(as_all[rbase:rbase + BS] + ad_all[rbase:rbase + BS])
            se = (ss * ms[:, None]).astype(BF).astype(np.float32)
            h_own = np.asarray(table[rbase:rbase + BS, :Fo], np.float32)
            hs = (h_own.reshape(BS, heads, HID)
                  * se[:, :, None]).astype(BF).astype(np.float32)
            agg += hs.reshape(BS, Fo)
            den += se
            o = agg.reshape(BS, heads, HID) / (den + 1e-16)[:, :, None]
            o = o.reshape(BS, Fo) + bias
            if relu:
                o = np.maximum(o, 0.0)
            out[rbase:rbase + BS] = o
    return out


def emulate(inputs, pp=None):
    """Full numpy emulation; returns [1, OUT_C]."""
    if pp is None:
        pp = preprocess(np.asarray(inputs["edge_index"]))
    x = np.asarray(inputs["x"], np.float32)
    W0e, W1e, W2e = host_weights(inputs)
    h = np.zeros((RTOT, IN_C), np.float32)
    valid = pp["xperm"] >= 0
    h[valid] = x[pp["xperm"][valid]]

    b0 = np.asarray(inputs["b0"], np.float32)
    b1 = np.asarray(inputs["b1"], np.float32)
    b2 = np.asarray(inputs["b2"], np.float32)

    h0 = _emulate_layer(h, pp, W0e, b0, HEADS, F1, True, EL01)
    h1 = _emulate_layer(h0, pp, W1e, b1, HEADS, F1, True, EL01)
    h2 = _emulate_layer(h1, pp, W2e, b2, 1, HID, False, EL2)

    g = h2[valid].sum(axis=0, keepdims=True) / N
    return (g @ np.asarray(inputs["hw"], np.float32)
            + np.asarray(inputs["hb"], np.float32)).astype(np.float32)


# ---------------- device kernel ----------------

_BUILT = None
_BUILT_KEY = None


def build_kernel(reg):
    import concourse.bacc as bacc
    import concourse.mybir as mybir
    import concourse.tile as tile
    from concourse import library_config

    f32 = mybir.dt.float32
    bf16 = mybir.dt.bfloat16
    i16 = mybir.dt.int16
    Alu = mybir.AluOpType
    Act = mybir.ActivationFunctionType

    nc = bacc.Bacc("TRN2", target_bir_lowering=False, debug=False,
                   num_devices=NCORES)

    # ---- I/O ----
    xtb_d = nc.dram_tensor("xtb", [NB, IN_C, BS], bf16, kind="ExternalInput")
    idx16_d = nc.dram_tensor("idx16", [128, NB * 2 * NKCOLS], i16,
                             kind="ExternalInput")
    dstc_d = nc.dram_tensor("dstc", [128, NB * 2 * TL], bf16,
                            kind="ExternalInput")
    dstr_d = nc.dram_tensor("dstr", [NB, KE2], bf16, kind="ExternalInput")
    maskc_d = nc.dram_tensor("maskc", [128, NB], bf16, kind="ExternalInput")
    mselfc_d = nc.dram_tensor("mselfc", [128, NB], bf16, kind="ExternalInput")
    w0e_d = nc.dram_tensor("w0e", [IN_C, F1 + 2 * HEADS], bf16,
                           kind="ExternalInput")
    w1e_d = nc.dram_tensor("w1e", [2, 128, F1 + 2 * HEADS], bf16,
                           kind="ExternalInput")
    w2e_d = nc.dram_tensor("w2e", [2, 128, HID + 2], bf16,
                           kind="ExternalInput")
    b0r_d = nc.dram_tensor("b0r", [128, F1], bf16, kind="ExternalInput")
    b1r_d = nc.dram_tensor("b1r", [128, F1], bf16, kind="ExternalInput")
    b2r_d = nc.dram_tensor("b2r", [128, HID], bf16, kind="ExternalInput")
    iota_row_d = nc.dram_tensor("iota_row", [128, 128], bf16,
                                kind="ExternalInput")
    iota_col_d = nc.dram_tensor("iota_col", [128, 1], bf16,
                                kind="ExternalInput")
    ones1_d = nc.dram_tensor("ones1", [1, 128], bf16, kind="ExternalInput")
    ident_d = nc.dram_tensor("ident", [128, 128], bf16, kind="ExternalInput")
    out_d = nc.dram_tensor("out_part", [1, OUT_C], f32, kind="ExternalOutput")

    # internal DRAM (Shared output for fast HBM-HBM AllGather)
    shardA = nc.dram_tensor("shardA", [NPC, EL01], bf16)
    tableA = nc.dram_tensor("tableA", [RTOT, EL01], bf16, addr_space="Shared")
    shardB = nc.dram_tensor("shardB", [NPC, EL01], bf16)
    tableB = nc.dram_tensor("tableB", [RTOT, EL01], bf16, addr_space="Shared")
    shard2 = nc.dram_tensor("shard2", [NPC, EL2], bf16)
    table2 = nc.dram_tensor("table2", [RTOT, EL2], bf16, addr_space="Shared")

    rg = [list(range(NCORES))]

    with tile.TileContext(nc) as tc:
        with (
            tc.tile_pool(name="const", bufs=1) as cpool,
            tc.tile_pool(name="gather", bufs=GBUFS) as gpool,
            tc.tile_pool(name="onehot", bufs=2) as mpool,
            tc.tile_pool(name="work", bufs=3) as wpool,
            tc.tile_pool(name="small", bufs=4) as spool,
            tc.tile_pool(name="adas", bufs=1) as apool,
            tc.tile_pool(name="ps_agg", bufs=2, space="PSUM") as ppagg,
            tc.tile_pool(name="ps_pad", bufs=2, space="PSUM") as pppad,
            tc.tile_pool(name="ps_rep", bufs=1, space="PSUM") as pprep,
            tc.tile_pool(name="ps_tp", bufs=1, space="PSUM") as pptp,
            tc.tile_pool(name="ps_tf", bufs=1, space="PSUM") as pptf,
            tc.tile_pool(name="ps_sum", bufs=1, space="PSUM") as ppsum,
        ):
            def load_const(tag, dram, shape, dtype=bf16, view=None):
                t = cpool.tile(shape, dtype, tag=tag)
                nc.sync.dma_start(out=t[:], in_=view if view is not None
                                  else dram[:])
                return t

            w0e_s = load_const("w0e", w0e_d, [IN_C, F1 + 2 * HEADS])
            w1e_s = load_const("w1e", w1e_d, [128, 2, F1 + 2 * HEADS],
                               view=w1e_d[:].rearrange("c p j -> p c j"))
            w2e_s = load_const("w2e", w2e_d, [128, 2, HID + 2],
                               view=w2e_d[:].rearrange("c p j -> p c j"))
            b0r_s = load_const("b0r", b0r_d, [128, F1])
            b1r_s = load_const("b1r", b1r_d, [128, F1])
            b2r_s = load_const("b2r", b2r_d, [128, HID])
            iota_row_s = load_const("iota_row", iota_row_d, [128, 128])
            iota_col_s = load_const("iota_col", iota_col_d, [128, 1])
            ones1_s = load_const("ones1", ones1_d, [1, 128])
            ident_s = load_const("ident", ident_d, [128, 128])
            idx16_s = load_const("idx16", idx16_d, [128, NB * 2 * NKCOLS], i16)
            dstc_s = load_const("dstc", dstc_d, [128, NB * 2 * TL])
            maskc_s = load_const("maskc", maskc_d, [128, NB])
            mselfc_s = load_const("mselfc", mselfc_d, [128, NB])

            nc.gpsimd.load_library(library_config.mlp)

            # persistent per-layer alpha tiles [128, NB*heads]
            as_all0 = apool.tile([128, NB * HEADS], bf16, tag="as0")
            as_all1 = apool.tile([128, NB * HEADS], bf16, tag="as1")
            as_all2 = apool.tile([128, NB], bf16, tag="as2")
            ad_all0 = apool.tile([128, NB * HEADS], bf16, tag="ad0")
            ad_all1 = apool.tile([128, NB * HEADS], bf16, tag="ad1")
            ad_all2 = apool.tile([128, NB], bf16, tag="ad2")
            as_all = [as_all0, as_all1, as_all2]
            ad_all = [ad_all0, ad_all1, ad_all2]

            LCFG = [  # heads, Fo, ncols, el, table, shard, bias, relu
                (HEADS, F1, F1 + 2 * HEADS, EL01, tableA, shardA, b0r_s, True),
                (HEADS, F1, F1 + 2 * HEADS, EL01, tableB, shardB, b1r_s, True),
                (1, HID, HID + 2, EL2, table2, shard2, b2r_s, False),
            ]

            def transform_block(layer, b, lhsT0, lhsT1):
                """Write table shard row-block b for `layer` from transposed
                activations (lhsT0/lhsT1 = [128 feat, 128 slot] bf16), or from
                xtb when layer == 0."""
                heads, Fo, ncols, el, _tb, shard, _bias, _relu = LCFG[layer]
                ps = pptf.tile([128, 512], f32, tag="tf", space="PSUM")
                if layer == 0:
                    nc.tensor.matmul(out=ps[:, :ncols], lhsT=lhsT0,
                                     rhs=w0e_s[:], start=True, stop=True)
                else:
                    we = w1e_s if layer == 1 else w2e_s
                    nc.tensor.matmul(out=ps[:, :ncols], lhsT=lhsT0,
                                     rhs=we[:, 0, :], start=True, stop=False)
                    nc.tensor.matmul(out=ps[:, :ncols], lhsT=lhsT1,
                                     rhs=we[:, 1, :], start=False, stop=True)
                tb = wpool.tile([128, EL01], bf16, tag="tb")
                nc.scalar.activation(tb[:, :ncols], ps[:, :ncols], Act.Copy)
                nc.vector.tensor_copy(
                    out=as_all[layer][:, b * heads:(b + 1) * heads],
                    in_=ps[:, Fo:Fo + heads])
                nc.vector.tensor_copy(
                    out=ad_all[layer][:, b * heads:(b + 1) * heads],
                    in_=ps[:, Fo + heads:Fo + 2 * heads])
                nc.sync.dma_start(out=shard[b * BS:(b + 1) * BS, :],
                                  in_=tb[:, :el])

            def allgather(layer):
                _h, _f, _n, _e, table, shard, _b, _r = LCFG[layer]
                nc.gpsimd.collective_compute(
                    "AllGather", mybir.AluOpType.bypass,
                    replica_groups=rg, ins=[shard[:].opt()],
                    outs=[table[:].opt()])

            def agg_block(layer, b):
                heads, Fo, ncols, el, table, shard, bias, relu = LCFG[layer]
                views = [table[0:LO_LIM, :], table[HI_BASE:HI_BASE + 32768, :]]

                # gathers (the critical Q7 stream) — issue first
                gtiles = []
                for kind in range(2):
                    g = gpool.tile([128, TL, el], bf16, tag="g")
                    nc.gpsimd.dma_gather(
                        g[:], views[kind],
                        idx16_s[:, (b * 2 + kind) * NKCOLS:
                                (b * 2 + kind + 1) * NKCOLS],
                        KE, int(reg[b][kind]), el,
                        single_packet=SINGLE_PACKET)
                    gtiles.append(g)

                # one-hot M [128e, (kind t) d]
                M = mpool.tile([128, TL2, 128], bf16, tag="M")
                nc.vector.tensor_tensor(
                    out=M[:],
                    in0=dstc_s[:, b * TL2:(b + 1) * TL2].unsqueeze(-1)
                        .broadcast_to([128, TL2, 128]),
                    in1=iota_row_s[:].unsqueeze(1)
                        .broadcast_to([128, TL2, 128]),
                    op=Alu.is_equal)
                # MT [128d, (kind t) e] via replicated-row outer product
                MT = mpool.tile([128, KE2], bf16, tag="MT")
                dr = spool.tile([1, KE2], bf16, tag="dr")
                nc.sync.dma_start(out=dr[:], in_=dstr_d[b:b + 1, :])
                for o in range(0, KE2, 512):
                    wd = min(512, KE2 - o)
                    pr = pprep.tile([128, 512], f32, tag="rep", space="PSUM")
                    nc.tensor.matmul(out=pr[:, :wd], lhsT=ones1_s[:],
                                     rhs=dr[:, o:o + wd],
                                     start=True, stop=True)
                    nc.vector.tensor_tensor(
                        out=MT[:, o:o + wd], in0=pr[:, :wd],
                        in1=iota_col_s[:].broadcast_to([128, wd]),
                        op=Alu.is_equal)

                # ad per edge: pad_[e, (kind t) h] = MT_t.T @ ad_block
                pad_ = pppad.tile([128, TL2 * heads], f32, tag="adp",
                                  space="PSUM")
                for t in range(TL2):
                    nc.tensor.matmul(
                        out=pad_[:, t * heads:(t + 1) * heads],
                        lhsT=MT[:, t * 128:(t + 1) * 128],
                        rhs=ad_all[layer][:, b * heads:(b + 1) * heads],
                        start=True, stop=True)

                # z = as + ad ; s = exp(lrelu(z)) into tmpS s-columns
                z = spool.tile([128, TL2 * heads], f32, tag="z")
                for kind in range(2):
                    nc.vector.tensor_tensor(
                        out=z[:, kind * TL * heads:(kind + 1) * TL * heads]
                            .rearrange("p (t h) -> p t h", t=TL),
                        in0=gtiles[kind][:, :, Fo:Fo + heads],
                        in1=pad_[:, kind * TL * heads:(kind + 1) * TL * heads]
                            .rearrange("p (t h) -> p t h", t=TL),
                        op=Alu.add)
                zl = spool.tile([128, TL2 * heads], f32, tag="zl")
                nc.scalar.activation(zl[:], z[:], Act.Lrelu, alpha=0.2)
                tmpS = wpool.tile([128, TL2, Fo + heads], bf16, tag="tmpS")
                nc.scalar.activation(
                    tmpS[:, :, Fo:Fo + heads],
                    zl[:].rearrange("p (t h) -> p t h", t=TL2), Act.Exp)
                # tmp = g * s (broadcast over HID), per (kind, head)
                sv = tmpS[:, :, Fo:Fo + heads]
                for kind in range(2):
                    for hh in range(heads):
                        nc.vector.tensor_tensor(
                            out=tmpS[:, kind * TL:(kind + 1) * TL,
                                     hh * HID:(hh + 1) * HID],
                            in0=gtiles[kind][:, :, hh * HID:(hh + 1) * HID],
                            in1=sv[:, kind * TL:(kind + 1) * TL, hh:hh + 1]
                                .broadcast_to([128, TL, HID]),
                            op=Alu.mult)

                # fused (agg | den) accumulation
                pagg = ppagg.tile([128, Fo + heads], f32, tag="agg",
                                  space="PSUM")
                for t in range(TL2):
                    nc.tensor.matmul(
                        out=pagg[:],
                        lhsT=M[:, t, :],
                        rhs=tmpS[:, t, :],
                        start=(t == 0), stop=(t == TL2 - 1))

                # self-loop contribution (local, not gathered)
                zs = spool.tile([128, heads], f32, tag="zs")
                nc.vector.tensor_tensor(
                    out=zs[:],
                    in0=as_all[layer][:, b * heads:(b + 1) * heads],
                    in1=ad_all[layer][:, b * heads:(b + 1) * heads],
                    op=Alu.add)
                zsl = spool.tile([128, heads], f32, tag="zsl")
                nc.scalar.activation(zsl[:], zs[:], Act.Lrelu, alpha=0.2)
                ses = spool.tile([128, heads], f32, tag="ses")
                nc.scalar.activation(ses[:], zsl[:], Act.Exp)
                se = spool.tile([128, heads], bf16, tag="se")
                nc.vector.tensor_tensor(
                    out=se[:], in0=ses[:],
                    in1=mselfc_s[:, b:b + 1].broadcast_to([128, heads]),
                    op=Alu.mult)
                h_own = wpool.tile([128, Fo], bf16, tag="hown")
                nc.sync.dma_start(out=h_own[:],
                                  in_=shard[b * BS:(b + 1) * BS, :Fo])
                hs = wpool.tile([128, Fo + heads], bf16, tag="hs")
                for hh in range(heads):
                    nc.vector.tensor_tensor(
                        out=hs[:, hh * HID:(hh + 1) * HID],
                        in0=h_own[:, hh * HID:(hh + 1) * HID],
                        in1=se[:, hh:hh + 1].broadcast_to([128, HID]),
                        op=Alu.mult)
                nc.vector.tensor_copy(out=hs[:, Fo:Fo + heads], in_=se[:])

                # epilogue
                t1 = wpool.tile([128, Fo + heads], f32, tag="t1")
                nc.vector.tensor_tensor(out=t1[:], in0=pagg[:], in1=hs[:],
                                        op=Alu.add)
                den = spool.tile([128, heads], f32, tag="den")
                nc.vector.tensor_scalar(out=den[:], in0=t1[:, Fo:Fo + heads],
                                        scalar1=1e-16, scalar2=None,
                                        op0=Alu.add)
                rec = spool.tile([128, heads], f32, tag="rec")
                nc.vector.reciprocal(out=rec[:], in_=den[:])
                o1 = wpool.tile([128, Fo], f32, tag="o1")
                nc.vector.tensor_tensor(
                    out=o1[:].rearrange("p (h f) -> p h f", h=heads),
                    in0=t1[:, :Fo].rearrange("p (h f) -> p h f", h=heads),
                    in1=rec[:].unsqueeze(-1).broadcast_to([128, heads, HID]),
                    op=Alu.mult)
                o2 = wpool.tile([128, Fo], bf16, tag="o2")
                nc.vector.tensor_tensor(out=o2[:], in0=o1[:],
                                        in1=bias[:, :Fo], op=Alu.add)
                if layer == 2:
                    return o2
                o3 = wpool.tile([128, Fo], bf16, tag="o3")
                nc.scalar.activation(o3[:], o2[:], Act.Relu)
                # transpose for next layer's transform
                hTb = []
                for k2 in range(2):
                    pt = pptp.tile([128, 128], bf16, tag="tp", space="PSUM")
                    nc.tensor.transpose(pt[:], o3[:, k2 * 128:(k2 + 1) * 128],
                                        ident_s[:])
                    ht = spool.tile([128, 128], bf16, tag=f"ht{k2}")
                    nc.vector.tensor_copy(out=ht[:], in_=pt[:])
                    hTb.append(ht)
                return hTb

            # ---- layer 0 transform (from host-provided x^T blocks) ----
            for b in range(NB):
                xb = wpool.tile([IN_C, BS], bf16, tag="xtb")
                nc.sync.dma_start(out=xb[:], in_=xtb_d[b])
                transform_block(0, b, xb[:], None)
            allgather(0)

            # prime gather tiles (stale-tail rows must be finite)
            for _ in range(GBUFS):
                g = gpool.tile([128, TL, EL01], bf16, tag="g")
                nc.vector.memset(g[:], 0.0)

            # ---- layer 0/1: aggregate + interleaved next transform ----
            for layer in range(2):
                for b in range(NB):
                    hTb = agg_block(layer, b)
                    transform_block(layer + 1, b, hTb[0][:], hTb[1][:])
                allgather(layer + 1)

            # ---- layer 2: aggregate + masked column sum ----
            psum_sum = ppsum.tile([1, OUT_C], f32, tag="sum", space="PSUM")
            for b in range(NB):
                o2 = agg_block(2, b)
                nc.tensor.matmul(out=psum_sum[:], lhsT=maskc_s[:, b:b + 1],
                                 rhs=o2[:], start=(b == 0),
                                 stop=(b == NB - 1))
            osb = spool.tile([1, OUT_C], f32, tag="osb")
            nc.vector.tensor_copy(out=osb[:], in_=psum_sum[:])
            nc.sync.dma_start(out=out_d[:], in_=osb[:])

    nc.compile()
    return nc


def _get_built(pp):
    global _BUILT, _BUILT_KEY
    key = pp["reg"].tobytes()
    if _BUILT is None or _BUILT_KEY != key:
        _BUILT = build_kernel([[int(pp["reg"][b][k]) for k in range(2)]
                               for b in range(NB)])
        _BUILT_KEY = key
    return _BUILT


def kernel(**inputs) -> np.ndarray:
    from concourse.bass_utils import run_bass_kernel_spmd

    pp = preprocess(np.asarray(inputs["edge_index"]))
    in_maps = build_core_inputs(inputs, pp)
    nc = _get_built(pp)
    res = run_bass_kernel_spmd(nc, in_maps, core_ids=list(range(NCORES)))
    parts = np.stack([r["out_part"][0] for r in res.results])  # [8, 64]
    g = parts.sum(axis=0, keepdims=True) / N
    out = (g @ np.asarray(inputs["hw"], np.float32)
           + np.asarray(inputs["hb"], np.float32)).astype(np.float32)
    return out


# revision 9
# speedup vs baseline: 1.3897x; 1.0620x over previous
"""3-layer GAT on 8 trn2 NeuronCores.

Strategy (graph/data parallel per sharding hint):
  - Nodes assigned to 8 cores x 49 blocks x 128 slots (degree-balanced LPT
    packing) -> permuted node order.  Table rows are chunk-major (4 chunks
    of blocks) so each chunk's AllGather can start as soon as its blocks'
    transforms are done, overlapping the collective with the block loop.
  - Per layer, per node block: transform with rhs = [W | W@as | W@ad] (alpha
    terms folded into the matmul) -> bf16 table shard chunk; AllGather
    (Shared output) -> full table on every core.
  - Aggregation per dst block: non-self edges (dst-sorted) packed into two
    int16-index windows (lo rows [0,32768), hi rows [17408,50176));
    dma_gather fetches the src rows; one-hot scatter matrices M [edge,dst]
    / MT [dst,edge] built on-device feed matmuls for the per-edge ad term
    and the fused (feature | denom) accumulation in PSUM.  Self-loop
    contributions are computed from the local shard and never gathered.
  - Softmax max-shift skipped (logits O(1), exp safe; mathematically equal).
  - Next layer's transform is interleaved into the aggregation loop
    (block-level pipeline); layer 2 reduces via a mask matmul; final mean +
    linear head on host.  ACT engine runs only Exp (no table churn); all
    casts/copies/lrelu/relu are DVE ops.
"""

import os
import numpy as np
import ml_dtypes

# ---------------- problem constants (must match reference) ----------------
N = 50000
E = 800000
IN_C = 128
HID = 64
HEADS = 4
OUT_C = 64
F1 = HEADS * HID  # 256

# ---------------- sharding geometry ----------------
NCORES = 8
NB = 49            # dst blocks per core
BS = 128           # dst slots per block
NPC = NB * BS      # 6272 nodes per core
RTOT = NCORES * NPC  # 50176 table rows
KE_CAP = 1152      # lo/hi packing capacity per (block, kind)
LO_LIM = 32768     # lo window rows [0, 32768)
HI_BASE = 17408    # hi window rows [17408, 50176)
CH = 4             # AllGather chunks (block ranges)
CHB = [0, 13, 25, 37, 49]  # chunk block boundaries

EL01 = 384         # table elems/row layers 0/1: 256 h + 4 as + 4 ad + pad
EL2 = 128          # table elems/row layer 2: 64 h + 1 as + 1 ad + pad
BF = ml_dtypes.bfloat16

GBUFS = 6          # gather tile double-buffer depth
SINGLE_PACKET = os.environ.get("GAT_SP", "0") == "1"

# chunk-major table row numbering:
#   chunk q holds blocks [CHB[q], CHB[q+1]) of every core; within chunk q,
#   rows are core-major.  row(c,b,s) = CBASE[q] + c*CROWS[q] + (b-CHB[q])*BS + s
CROWS = [(CHB[q + 1] - CHB[q]) * BS for q in range(CH)]
CBASE = [0]
for q in range(CH):
    CBASE.append(CBASE[-1] + NCORES * CROWS[q])
assert CBASE[-1] == RTOT


def _chunk_of_block(b):
    for q in range(CH):
        if CHB[q] <= b < CHB[q + 1]:
            return q
    raise ValueError(b)


def _row_base(c, b):
    q = _chunk_of_block(b)
    return CBASE[q] + c * CROWS[q] + (b - CHB[q]) * BS


# ---------------- host preprocessing ----------------

def preprocess(edge_index):
    """Node->(core,block,slot) assignment and per-core edge tile arrays."""
    import heapq

    e0 = np.asarray(edge_index[0], np.int64)
    e1 = np.asarray(edge_index[1], np.int64)
    nonself = e0 != e1
    src = e0[nonself]
    dst = e1[nonself]
    # self-edge multiplicity: 1 (PyG added loop) + natural self edges
    mult = np.ones(N, np.int64)
    np.add.at(mult, e0[~nonself], 1)

    deg = np.bincount(dst, minlength=N)  # gather load per dst node

    nblocks = NCORES * NB
    order = np.argsort(-deg, kind="stable")
    heap = [(0, b) for b in range(nblocks)]
    heapq.heapify(heap)
    slots_used = np.zeros(nblocks, np.int64)
    node_block = np.empty(N, np.int64)
    node_slot = np.empty(N, np.int64)
    for n in order:
        while True:
            load, b = heapq.heappop(heap)
            if slots_used[b] < BS:
                break
        node_block[n] = b
        node_slot[n] = slots_used[b]
        slots_used[b] += 1
        heapq.heappush(heap, (load + int(deg[n]), b))

    # chunk-major global table row of each node
    rbase_tab = np.empty((NCORES, NB), np.int64)
    for c in range(NCORES):
        for bl in range(NB):
            rbase_tab[c, bl] = _row_base(c, bl)
    node_core, node_bl = np.divmod(node_block, NB)
    row = rbase_tab[node_core, node_bl] + node_slot

    xperm = np.full(RTOT, -1, np.int64)
    xperm[row] = np.arange(N)

    erow = row[src]
    eblk = node_block[dst]
    eslot = node_slot[dst]

    order_e = np.argsort(eblk, kind="stable")
    bounds = np.searchsorted(eblk[order_e], np.arange(nblocks + 1))

    # pass 1: split lo/hi, count
    packed = {}
    cnt = np.zeros((NCORES, NB, 2), np.int64)
    for b in range(nblocks):
        c, bl = divmod(b, NB)
        es = order_e[bounds[b]:bounds[b + 1]]
        r_ = erow[es]
        dl = eslot[es]
        lo_f = r_ < HI_BASE
        hi_f = r_ >= LO_LIM
        flex = ~lo_f & ~hi_f
        n_lo = int(lo_f.sum())
        n_hi = int(hi_f.sum())
        n_fx = int(flex.sum())
        tot = n_lo + n_hi + n_fx
        assert tot <= 2 * KE_CAP, f"block {b} has {tot} edges > {2*KE_CAP}"
        add_lo = min(n_fx, max(0, min(KE_CAP, (tot + 1) // 2) - n_lo))
        if n_hi + (n_fx - add_lo) > KE_CAP:
            add_lo = n_fx - (KE_CAP - n_hi)
        assert 0 <= add_lo <= n_fx
        fx_idx = np.nonzero(flex)[0]
        sel_lo = np.zeros(len(es), bool)
        sel_lo[lo_f] = True
        sel_lo[fx_idx[:add_lo]] = True
        for kind, sel, base in ((0, sel_lo, 0), (1, ~sel_lo, HI_BASE)):
            rr = r_[sel]
            dd = dl[sel]
            o = np.argsort(rr, kind="stable")  # DMA locality
            packed[(b, kind)] = (rr[o] - base, dd[o])
            cnt[c, bl, kind] = len(rr)

    # static per-(block,kind) gather count = max over cores, 16-aligned;
    # tile count TL derived from the largest count
    reg = cnt.max(axis=0)
    reg = ((reg + 15) // 16 * 16).astype(np.int64)
    tl = int(np.ceil(reg.max() / 128))
    ke = tl * 128
    nkcols = ke // 16
    assert (reg <= ke).all() and ke <= KE_CAP

    idx16 = np.full((NCORES, 128, NB * 2 * nkcols), -1, np.int16)
    dstc = np.full((NCORES, 128, NB * 2 * tl), -1.0, np.float32)
    dstr = np.full((NCORES, NB, 2 * ke), -1.0, np.float32)
    maskc = np.zeros((NCORES, 128, NB), np.float32)
    mselfc = np.zeros((NCORES, 128, NB), np.float32)

    # pass 2: pack indices (pad to reg with row 0, -1 beyond)
    for b in range(nblocks):
        c, bl = divmod(b, NB)
        for kind in range(2):
            rel, dd = packed[(b, kind)]
            k = len(rel)
            r = int(reg[bl, kind])
            full = np.full(ke, -1, np.int64)
            full[:k] = rel
            full[k:r] = 0  # safe real row, contributes nothing (dst=-1)
            assert k == 0 or (rel.min() >= 0 and rel.max() < 32768)
            w = full.reshape(nkcols, 16).T.astype(np.int16)  # [16, nkcols]
            cbase = (bl * 2 + kind) * nkcols
            idx16[c, :, cbase:cbase + nkcols] = np.tile(w, (8, 1))
            dloc = np.full(ke, -1.0, np.float32)
            dloc[:k] = dd.astype(np.float32)
            tcol = bl * 2 * tl + kind * tl
            dstc[c, :, tcol:tcol + tl] = dloc.reshape(tl, 128).T
            dstr[c, bl, kind * ke:(kind + 1) * ke] = dloc

        used = slots_used[b]
        maskc[c, :used, bl] = 1.0
        nodes_b = np.where(node_block == b)[0]
        mselfc[c, node_slot[nodes_b], bl] = mult[nodes_b].astype(np.float32)

    return dict(row=row, xperm=xperm, idx16=idx16, dstc=dstc, dstr=dstr,
                maskc=maskc, mselfc=mselfc, cnt=cnt, reg=reg, tl=tl,
                node_block=node_block, node_slot=node_slot)


def host_weights(inputs):
    """Extended weight matrices with folded attention vectors."""
    def ext(W, a_s, a_d, heads):
        Wh = W.reshape(W.shape[0], heads, HID)
        Was = np.einsum("khc,hc->kh", Wh, a_s)
        Wad = np.einsum("khc,hc->kh", Wh, a_d)
        return np.concatenate([W, Was, Wad], axis=1).astype(np.float32)

    W0e = ext(np.asarray(inputs["W0"], np.float32),
              np.asarray(inputs["a0s"], np.float32),
              np.asarray(inputs["a0d"], np.float32), HEADS)      # [128, 264]
    W1e = ext(np.asarray(inputs["W1"], np.float32),
              np.asarray(inputs["a1s"], np.float32),
              np.asarray(inputs["a1d"], np.float32), HEADS)      # [256, 264]
    W2e = ext(np.asarray(inputs["W2"], np.float32),
              np.asarray(inputs["a2s"], np.float32),
              np.asarray(inputs["a2d"], np.float32), 1)          # [256, 66]
    return W0e, W1e, W2e


def build_core_inputs(inputs, pp):
    """Per-core in_maps for run_bass_kernel_spmd."""
    x = np.asarray(inputs["x"], np.float32)
    W0e, W1e, W2e = host_weights(inputs)
    b0 = np.asarray(inputs["b0"], np.float32)
    b1 = np.asarray(inputs["b1"], np.float32)
    b2 = np.asarray(inputs["b2"], np.float32)

    iota_row = np.tile(np.arange(128, dtype=np.float32), (128, 1))
    iota_col = np.arange(128, dtype=np.float32).reshape(128, 1)
    ones1 = np.ones((1, 128), np.float32)
    ident = np.eye(128, dtype=np.float32)

    consts = dict(
        w0e=W0e.astype(BF),
        w1e=W1e.reshape(2, 128, F1 + 2 * HEADS).astype(BF),
        w2e=W2e.reshape(2, 128, HID + 2).astype(BF),
        b0r=np.tile(b0, (128, 1)).astype(BF),
        b1r=np.tile(b1, (128, 1)).astype(BF),
        b2r=np.tile(b2, (128, 1)).astype(BF),
        iota_row=iota_row.astype(BF), iota_col=iota_col.astype(BF),
        ones1=ones1.astype(BF), ident=ident.astype(BF),
    )

    in_maps = []
    for c in range(NCORES):
        xtb = np.zeros((NB, IN_C, BS), np.float32)
        for b in range(NB):
            rb = _row_base(c, b)
            nb = pp["xperm"][rb:rb + BS]
            valid = nb >= 0
            if valid.any():
                xtb[b][:, valid] = x[nb[valid]].T
        m = dict(
            xtb=xtb.astype(BF),
            idx16=pp["idx16"][c],
            dstc=pp["dstc"][c].astype(BF),
            dstr=pp["dstr"][c].astype(BF),
            maskc=pp["maskc"][c].astype(BF),
            mselfc=pp["mselfc"][c].astype(BF),
            **consts,
        )
        in_maps.append(m)
    return in_maps


# ---------------- numpy emulation of the device data path ----------------

def _emulate_layer(tables_in, pp, We, bias, heads, Fo, relu, el):
    """tables_in: node-major feature mat [RTOT, F_in] (f32).
    Returns out [RTOT, Fo] node-major post-activation."""
    tl = pp["tl"]
    ke = tl * 128
    nkcols = ke // 16
    ncols = Fo + 2 * heads
    tb = (tables_in.astype(BF).astype(np.float32)
          @ We.astype(BF).astype(np.float32))
    table = np.zeros((RTOT, el), BF)
    table[:, :ncols] = tb.astype(BF)
    as_all = tb[:, Fo:Fo + heads].astype(BF).astype(np.float32)
    ad_all = tb[:, Fo + heads:Fo + 2 * heads].astype(BF).astype(np.float32)

    def lrexp(z):
        return np.exp(np.maximum(z, 0.2 * z)).astype(np.float32)

    out = np.zeros((RTOT, Fo), np.float32)
    for c in range(NCORES):
        for bl in range(NB):
            rbase = _row_base(c, bl)
            agg = np.zeros((BS, Fo), np.float32)
            den = np.zeros((BS, heads), np.float32)
            for kind in range(2):
                base = 0 if kind == 0 else HI_BASE
                cbase = (bl * 2 + kind) * nkcols
                w = pp["idx16"][c][:16, cbase:cbase + nkcols]
                rel = w.T.reshape(-1).astype(np.int64)
                r = int(pp["reg"][bl, kind])
                rows = rel[:r] + base
                g = np.asarray(table[rows], np.float32)  # [r, el]
                dl = pp["dstr"][c][bl, kind * ke:kind * ke + r].astype(np.int64)
                valid = dl >= 0
                a_s = g[:, Fo:Fo + heads]
                a_d = np.where(valid[:, None], ad_all[rbase + dl], 0.0)
                s = lrexp(a_s + a_d).astype(BF).astype(np.float32)
                hsc = (g[:, :Fo].reshape(r, heads, HID)
                       * s[:, :, None]).astype(BF).astype(np.float32)
                hsc = hsc.reshape(r, Fo)
                np.add.at(agg, dl[valid], hsc[valid])
                np.add.at(den, dl[valid], s[valid])
            # self loops
            ms = pp["mselfc"][c][:, bl]  # [BS]
            ss = lrexp(as_all[rbase:rbase + BS] + ad_all[rbase:rbase + BS])
            se = (ss * ms[:, None]).astype(BF).astype(np.float32)
            h_own = np.asarray(table[rbase:rbase + BS, :Fo], np.float32)
            hs = (h_own.reshape(BS, heads, HID)
                  * se[:, :, None]).astype(BF).astype(np.float32)
            agg += hs.reshape(BS, Fo)
            den += se
            o = agg.reshape(BS, heads, HID) / (den + 1e-16)[:, :, None]
            o = o.reshape(BS, Fo) + bias
            if relu:
                o = np.maximum(o, 0.0)
            out[rbase:rbase + BS] = o
    return out


def emulate(inputs, pp=None):
    """Full numpy emulation; returns [1, OUT_C]."""
    if pp is None:
        pp = preprocess(np.asarray(inputs["edge_index"]))
    x = np.asarray(inputs["x"], np.float32)
    W0e, W1e, W2e = host_weights(inputs)
    h = np.zeros((RTOT, IN_C), np.float32)
    valid = pp["xperm"] >= 0
    h[valid] = x[pp["xperm"][valid]]

    b0 = np.asarray(inputs["b0"], np.float32)
    b1 = np.asarray(inputs["b1"], np.float32)
    b2 = np.asarray(inputs["b2"], np.float32)

    h0 = _emulate_layer(h, pp, W0e, b0, HEADS, F1, True, EL01)
    h1 = _emulate_layer(h0, pp, W1e, b1, HEADS, F1, True, EL01)
    h2 = _emulate_layer(h1, pp, W2e, b2, 1, HID, False, EL2)

    g = h2[valid].sum(axis=0, keepdims=True) / N
    return (g @ np.asarray(inputs["hw"], np.float32)
            + np.asarray(inputs["hb"], np.float32)).astype(np.float32)


# ---------------- device kernel ----------------

_BUILT = None
_BUILT_KEY = None


def build_kernel(reg, tl):
    import concourse.bacc as bacc
    import concourse.mybir as mybir
    import concourse.tile as tile
    from concourse import library_config

    ke = tl * 128
    tl2 = 2 * tl
    ke2 = 2 * ke
    nkcols = ke // 16

    f32 = mybir.dt.float32
    bf16 = mybir.dt.bfloat16
    i16 = mybir.dt.int16
    Alu = mybir.AluOpType
    Act = mybir.ActivationFunctionType

    nc = bacc.Bacc("TRN2", target_bir_lowering=False, debug=False,
                   num_devices=NCORES)

    # ---- I/O ----
    xtb_d = nc.dram_tensor("xtb", [NB, IN_C, BS], bf16, kind="ExternalInput")
    idx16_d = nc.dram_tensor("idx16", [128, NB * 2 * nkcols], i16,
                             kind="ExternalInput")
    dstc_d = nc.dram_tensor("dstc", [128, NB * 2 * tl], bf16,
                            kind="ExternalInput")
    dstr_d = nc.dram_tensor("dstr", [NB, ke2], bf16, kind="ExternalInput")
    maskc_d = nc.dram_tensor("maskc", [128, NB], bf16, kind="ExternalInput")
    mselfc_d = nc.dram_tensor("mselfc", [128, NB], bf16, kind="ExternalInput")
    w0e_d = nc.dram_tensor("w0e", [IN_C, F1 + 2 * HEADS], bf16,
                           kind="ExternalInput")
    w1e_d = nc.dram_tensor("w1e", [2, 128, F1 + 2 * HEADS], bf16,
                           kind="ExternalInput")
    w2e_d = nc.dram_tensor("w2e", [2, 128, HID + 2], bf16,
                           kind="ExternalInput")
    b0r_d = nc.dram_tensor("b0r", [128, F1], bf16, kind="ExternalInput")
    b1r_d = nc.dram_tensor("b1r", [128, F1], bf16, kind="ExternalInput")
    b2r_d = nc.dram_tensor("b2r", [128, HID], bf16, kind="ExternalInput")
    iota_row_d = nc.dram_tensor("iota_row", [128, 128], bf16,
                                kind="ExternalInput")
    iota_col_d = nc.dram_tensor("iota_col", [128, 1], bf16,
                                kind="ExternalInput")
    ones1_d = nc.dram_tensor("ones1", [1, 128], bf16, kind="ExternalInput")
    ident_d = nc.dram_tensor("ident", [128, 128], bf16, kind="ExternalInput")
    out_d = nc.dram_tensor("out_part", [1, OUT_C], f32, kind="ExternalOutput")

    # internal DRAM: per-layer tables (Shared) + per-chunk shards
    tables = []
    shards = []
    for li, el in enumerate([EL01, EL01, EL2]):
        tables.append(nc.dram_tensor(f"table{li}", [RTOT, el], bf16,
                                     addr_space="Shared"))
        shards.append([nc.dram_tensor(f"shard{li}_{q}", [CROWS[q], el], bf16)
                       for q in range(CH)])

    rg = [list(range(NCORES))]

    with tile.TileContext(nc) as tc:
        with (
            tc.tile_pool(name="const", bufs=1) as cpool,
            tc.tile_pool(name="gather", bufs=GBUFS) as gpool,
            tc.tile_pool(name="onehot", bufs=2) as mpool,
            tc.tile_pool(name="work", bufs=3) as wpool,
            tc.tile_pool(name="small", bufs=4) as spool,
            tc.tile_pool(name="adas", bufs=1) as apool,
            tc.tile_pool(name="ps_agg", bufs=2, space="PSUM") as ppagg,
            tc.tile_pool(name="ps_pad", bufs=2, space="PSUM") as pppad,
            tc.tile_pool(name="ps_rep", bufs=1, space="PSUM") as pprep,
            tc.tile_pool(name="ps_tp", bufs=1, space="PSUM") as pptp,
            tc.tile_pool(name="ps_tf", bufs=1, space="PSUM") as pptf,
            tc.tile_pool(name="ps_sum", bufs=1, space="PSUM") as ppsum,
        ):
            def load_const(tag, dram, shape, dtype=bf16, view=None):
                t = cpool.tile(shape, dtype, tag=tag)
                nc.sync.dma_start(out=t[:], in_=view if view is not None
                                  else dram[:])
                return t

            w0e_s = load_const("w0e", w0e_d, [IN_C, F1 + 2 * HEADS])
            w1e_s = load_const("w1e", w1e_d, [128, 2, F1 + 2 * HEADS],
                               view=w1e_d[:].rearrange("c p j -> p c j"))
            w2e_s = load_const("w2e", w2e_d, [128, 2, HID + 2],
                               view=w2e_d[:].rearrange("c p j -> p c j"))
            b0r_s = load_const("b0r", b0r_d, [128, F1])
            b1r_s = load_const("b1r", b1r_d, [128, F1])
            b2r_s = load_const("b2r", b2r_d, [128, HID])
            iota_row_s = load_const("iota_row", iota_row_d, [128, 128])
            iota_col_s = load_const("iota_col", iota_col_d, [128, 1])
            ones1_s = load_const("ones1", ones1_d, [1, 128])
            ident_s = load_const("ident", ident_d, [128, 128])
            idx16_s = load_const("idx16", idx16_d, [128, NB * 2 * nkcols],
                                 i16)
            dstc_s = load_const("dstc", dstc_d, [128, NB * 2 * tl])
            maskc_s = load_const("maskc", maskc_d, [128, NB])
            mselfc_s = load_const("mselfc", mselfc_d, [128, NB])

            nc.gpsimd.load_library(library_config.mlp)

            # persistent per-layer alpha tiles [128, NB*heads]
            as_all0 = apool.tile([128, NB * HEADS], bf16, tag="as0")
            as_all1 = apool.tile([128, NB * HEADS], bf16, tag="as1")
            as_all2 = apool.tile([128, NB], bf16, tag="as2")
            ad_all0 = apool.tile([128, NB * HEADS], bf16, tag="ad0")
            ad_all1 = apool.tile([128, NB * HEADS], bf16, tag="ad1")
            ad_all2 = apool.tile([128, NB], bf16, tag="ad2")
            as_all = [as_all0, as_all1, as_all2]
            ad_all = [ad_all0, ad_all1, ad_all2]

            LCFG = [  # heads, Fo, ncols, el, bias, relu
                (HEADS, F1, F1 + 2 * HEADS, EL01, b0r_s, True),
                (HEADS, F1, F1 + 2 * HEADS, EL01, b1r_s, True),
                (1, HID, HID + 2, EL2, b2r_s, False),
            ]

            def transform_block(layer, b, lhsT0, lhsT1):
                """Write table-shard rows of block b for `layer` from
                transposed activations ([128 feat, 128 slot] bf16), or from
                xtb when layer == 0."""
                heads, Fo, ncols, el, _bias, _relu = LCFG[layer]
                q = _chunk_of_block(b)
                shard = shards[layer][q]
                roff = (b - CHB[q]) * BS
                ps = pptf.tile([128, 512], f32, tag="tf", space="PSUM")
                if layer == 0:
                    nc.tensor.matmul(out=ps[:, :ncols], lhsT=lhsT0,
                                     rhs=w0e_s[:], start=True, stop=True)
                else:
                    we = w1e_s if layer == 1 else w2e_s
                    nc.tensor.matmul(out=ps[:, :ncols], lhsT=lhsT0,
                                     rhs=we[:, 0, :], start=True, stop=False)
                    nc.tensor.matmul(out=ps[:, :ncols], lhsT=lhsT1,
                                     rhs=we[:, 1, :], start=False, stop=True)
                tb = wpool.tile([128, EL01], bf16, tag="tb")
                nc.vector.tensor_copy(out=tb[:, :ncols], in_=ps[:, :ncols])
                nc.vector.tensor_copy(
                    out=as_all[layer][:, b * heads:(b + 1) * heads],
                    in_=ps[:, Fo:Fo + heads])
                nc.vector.tensor_copy(
                    out=ad_all[layer][:, b * heads:(b + 1) * heads],
                    in_=ps[:, Fo + heads:Fo + 2 * heads])
                nc.sync.dma_start(out=shard[roff:roff + BS, :],
                                  in_=tb[:, :el])

            def allgather(layer, q):
                el = LCFG[layer][3]
                table = tables[layer]
                nc.gpsimd.collective_compute(
                    "AllGather", mybir.AluOpType.bypass,
                    replica_groups=rg, ins=[shards[layer][q][:].opt()],
                    outs=[table[CBASE[q]:CBASE[q + 1], :].opt()])

            def agg_block(layer, b):
                heads, Fo, ncols, el, bias, relu = LCFG[layer]
                table = tables[layer]
                q = _chunk_of_block(b)
                shard = shards[layer][q]
                roff = (b - CHB[q]) * BS
                views = [table[0:LO_LIM, :], table[HI_BASE:HI_BASE + 32768, :]]

                # gathers (the critical Q7 stream) — issue first
                gtiles = []
                for kind in range(2):
                    g = gpool.tile([128, tl, el], bf16, tag="g")
                    nc.gpsimd.dma_gather(
                        g[:], views[kind],
                        idx16_s[:, (b * 2 + kind) * nkcols:
                                (b * 2 + kind + 1) * nkcols],
                        ke, int(reg[b][kind]), el,
                        single_packet=SINGLE_PACKET)
                    gtiles.append(g)

                # one-hot M [128e, (kind t) d]
                M = mpool.tile([128, tl2, 128], bf16, tag="M")
                nc.vector.tensor_tensor(
                    out=M[:],
                    in0=dstc_s[:, b * tl2:(b + 1) * tl2].unsqueeze(-1)
                        .broadcast_to([128, tl2, 128]),
                    in1=iota_row_s[:].unsqueeze(1)
                        .broadcast_to([128, tl2, 128]),
                    op=Alu.is_equal)
                # MT [128d, (kind t) e] via replicated-row outer product
                MT = mpool.tile([128, ke2], bf16, tag="MT")
                dr = spool.tile([1, ke2], bf16, tag="dr")
                nc.sync.dma_start(out=dr[:], in_=dstr_d[b:b + 1, :])
                for o in range(0, ke2, 512):
                    wd = min(512, ke2 - o)
                    pr = pprep.tile([128, 512], f32, tag="rep", space="PSUM")
                    nc.tensor.matmul(out=pr[:, :wd], lhsT=ones1_s[:],
                                     rhs=dr[:, o:o + wd],
                                     start=True, stop=True)
                    nc.vector.tensor_tensor(
                        out=MT[:, o:o + wd], in0=pr[:, :wd],
                        in1=iota_col_s[:].broadcast_to([128, wd]),
                        op=Alu.is_equal)

                # ad per edge: pad_[e, (kind t) h] = MT_t.T @ ad_block
                pad_ = pppad.tile([128, tl2 * heads], f32, tag="adp",
                                  space="PSUM")
                for t in range(tl2):
                    nc.tensor.matmul(
                        out=pad_[:, t * heads:(t + 1) * heads],
                        lhsT=MT[:, t * 128:(t + 1) * 128],
                        rhs=ad_all[layer][:, b * heads:(b + 1) * heads],
                        start=True, stop=True)

                # z = as + ad for edges, plus self z in the tail columns
                nzc = tl2 * heads
                z = spool.tile([128, nzc + heads], f32, tag="z")
                for kind in range(2):
                    nc.vector.tensor_tensor(
                        out=z[:, kind * tl * heads:(kind + 1) * tl * heads]
                            .rearrange("p (t h) -> p t h", t=tl),
                        in0=gtiles[kind][:, :, Fo:Fo + heads],
                        in1=pad_[:, kind * tl * heads:(kind + 1) * tl * heads]
                            .rearrange("p (t h) -> p t h", t=tl),
                        op=Alu.add)
                nc.vector.tensor_tensor(
                    out=z[:, nzc:nzc + heads],
                    in0=as_all[layer][:, b * heads:(b + 1) * heads],
                    in1=ad_all[layer][:, b * heads:(b + 1) * heads],
                    op=Alu.add)
                # lrelu on DVE: zl = max(0.2*z, z)
                zl = spool.tile([128, nzc + heads], f32, tag="zl")
                nc.vector.scalar_tensor_tensor(
                    out=zl[:], in0=z[:], scalar=0.2, in1=z[:],
                    op0=Alu.mult, op1=Alu.max)
                # s = exp(zl): edge s into tmpS tail cols, self s into ses
                tmpS = wpool.tile([128, tl2, Fo + heads], bf16, tag="tmpS")
                nc.scalar.activation(
                    tmpS[:, :, Fo:Fo + heads],
                    zl[:, :nzc].rearrange("p (t h) -> p t h", t=tl2), Act.Exp)
                ses = spool.tile([128, heads], f32, tag="ses")
                nc.scalar.activation(ses[:], zl[:, nzc:nzc + heads], Act.Exp)
                # tmp = g * s (broadcast over HID), per (kind, head)
                sv = tmpS[:, :, Fo:Fo + heads]
                for kind in range(2):
                    for hh in range(heads):
                        nc.vector.tensor_tensor(
                            out=tmpS[:, kind * tl:(kind + 1) * tl,
                                     hh * HID:(hh + 1) * HID],
                            in0=gtiles[kind][:, :, hh * HID:(hh + 1) * HID],
                            in1=sv[:, kind * tl:(kind + 1) * tl, hh:hh + 1]
                                .broadcast_to([128, tl, HID]),
                            op=Alu.mult)

                # fused (agg | den) accumulation
                pagg = ppagg.tile([128, Fo + heads], f32, tag="agg",
                                  space="PSUM")
                for t in range(tl2):
                    nc.tensor.matmul(
                        out=pagg[:],
                        lhsT=M[:, t, :],
                        rhs=tmpS[:, t, :],
                        start=(t == 0), stop=(t == tl2 - 1))

                # self-loop contribution (local shard, not gathered)
                se = spool.tile([128, heads], bf16, tag="se")
                nc.vector.tensor_tensor(
                    out=se[:], in0=ses[:],
                    in1=mselfc_s[:, b:b + 1].broadcast_to([128, heads]),
                    op=Alu.mult)
                h_own = wpool.tile([128, Fo], bf16, tag="hown")
                nc.sync.dma_start(out=h_own[:],
                                  in_=shard[roff:roff + BS, :Fo])
                hs = wpool.tile([128, Fo + heads], bf16, tag="hs")
                for hh in range(heads):
                    nc.vector.tensor_tensor(
                        out=hs[:, hh * HID:(hh + 1) * HID],
                        in0=h_own[:, hh * HID:(hh + 1) * HID],
                        in1=se[:, hh:hh + 1].broadcast_to([128, HID]),
                        op=Alu.mult)
                nc.vector.tensor_copy(out=hs[:, Fo:Fo + heads], in_=se[:])

                # epilogue
                t1 = wpool.tile([128, Fo + heads], f32, tag="t1")
                nc.vector.tensor_tensor(out=t1[:], in0=pagg[:], in1=hs[:],
                                        op=Alu.add)
                den = spool.tile([128, heads], f32, tag="den")
                nc.vector.tensor_scalar(out=den[:], in0=t1[:, Fo:Fo + heads],
                                        scalar1=1e-16, scalar2=None,
                                        op0=Alu.add)
                rec = spool.tile([128, heads], f32, tag="rec")
                nc.vector.reciprocal(out=rec[:], in_=den[:])
                o1 = wpool.tile([128, Fo], f32, tag="o1")
                nc.vector.tensor_tensor(
                    out=o1[:].rearrange("p (h f) -> p h f", h=heads),
                    in0=t1[:, :Fo].rearrange("p (h f) -> p h f", h=heads),
                    in1=rec[:].unsqueeze(-1).broadcast_to([128, heads, HID]),
                    op=Alu.mult)
                o2 = wpool.tile([128, Fo], bf16, tag="o2")
                nc.vector.tensor_tensor(out=o2[:], in0=o1[:],
                                        in1=bias[:, :Fo], op=Alu.add)
                if layer == 2:
                    return o2
                # relu on DVE (keeps ACT exp-only)
                o3 = wpool.tile([128, Fo], bf16, tag="o3")
                nc.vector.tensor_scalar(out=o3[:], in0=o2[:],
                                        scalar1=0.0, scalar2=None,
                                        op0=Alu.max)
                # transpose for next layer's transform
                hTb = []
                for k2 in range(2):
                    pt = pptp.tile([128, 128], bf16, tag="tp", space="PSUM")
                    nc.tensor.transpose(pt[:], o3[:, k2 * 128:(k2 + 1) * 128],
                                        ident_s[:])
                    ht = spool.tile([128, 128], bf16, tag=f"ht{k2}")
                    nc.vector.tensor_copy(out=ht[:], in_=pt[:])
                    hTb.append(ht)
                return hTb

            # ---- layer 0 transform (from host-provided x^T blocks) ----
            for b in range(NB):
                xb = wpool.tile([IN_C, BS], bf16, tag="xtb")
                nc.sync.dma_start(out=xb[:], in_=xtb_d[b])
                transform_block(0, b, xb[:], None)
                if b + 1 in CHB:
                    allgather(0, CHB.index(b + 1) - 1)

            # prime gather tiles (stale-tail rows must be finite)
            for _ in range(GBUFS):
                g = gpool.tile([128, tl, EL01], bf16, tag="g")
                nc.vector.memset(g[:], 0.0)

            # ---- layer 0/1: aggregate + interleaved next transform ----
            for layer in range(2):
                for b in range(NB):
                    hTb = agg_block(layer, b)
                    transform_block(layer + 1, b, hTb[0][:], hTb[1][:])
                    if b + 1 in CHB:
                        allgather(layer + 1, CHB.index(b + 1) - 1)

            # ---- layer 2: aggregate + masked column sum ----
            psum_sum = ppsum.tile([1, OUT_C], f32, tag="sum", space="PSUM")
            for b in range(NB):
                o2 = agg_block(2, b)
                nc.tensor.matmul(out=psum_sum[:], lhsT=maskc_s[:, b:b + 1],
                                 rhs=o2[:], start=(b == 0),
                                 stop=(b == NB - 1))
            osb = spool.tile([1, OUT_C], f32, tag="osb")
            nc.vector.tensor_copy(out=osb[:], in_=psum_sum[:])
            nc.sync.dma_start(out=out_d[:], in_=osb[:])

    nc.compile()
    return nc


def _get_built(pp):
    global _BUILT, _BUILT_KEY
    key = (pp["reg"].tobytes(), pp["tl"])
    if _BUILT is None or _BUILT_KEY != key:
        _BUILT = build_kernel([[int(pp["reg"][b][k]) for k in range(2)]
                               for b in range(NB)], pp["tl"])
        _BUILT_KEY = key
    return _BUILT


def kernel(**inputs) -> np.ndarray:
    from concourse.bass_utils import run_bass_kernel_spmd

    pp = preprocess(np.asarray(inputs["edge_index"]))
    in_maps = build_core_inputs(inputs, pp)
    nc = _get_built(pp)
    res = run_bass_kernel_spmd(nc, in_maps, core_ids=list(range(NCORES)))
    parts = np.stack([r["out_part"][0] for r in res.results])  # [8, 64]
    g = parts.sum(axis=0, keepdims=True) / N
    out = (g @ np.asarray(inputs["hw"], np.float32)
           + np.asarray(inputs["hb"], np.float32)).astype(np.float32)
    return out
